# revision 1
# baseline (speedup 1.0000x reference)
"""Trainium2 Bass kernel for GQA attention (B=2, S=2048, D=2048, H=16, KVH=4).

Sharding: 8 cores = (batch b in {0,1}) x (kv-group g in {0..3}).
Core c = b*4 + g computes q-heads 4g..4g+3 against kv-head g for batch b,
producing a partial output projection res_partial.T = [e=2048, s=2048];
host sums the 4 partials per batch.

Device layout notes (per core):
  - All matmuls in fp32r (TF32-like, full PE rate at N>=512).
  - qT/kT layout: [head_dim on partitions, seq on free] -> QK^T and PV need no
    transposes; softmax denominator via ones-column matmul on the PE.
  - scores computed transposed: S.T[k_pos, q_pos]; exp on ACT without
    max-subtraction (|score| <= sqrt(128), safe for gain==1);
    causal mask via gpsimd affine_select on diagonal blocks.
  - RMS-norm partition sums + 1/rms broadcast via tiny PE matmuls;
    rsqrt/reciprocal on ACT (Abs_reciprocal_sqrt, Square) - DVE reciprocal
    is 3.3us/row, the ACT table is 4e-5 accurate.
"""

import sys

sys.path.insert(0, "/opt/trn_rl_repo")

from contextlib import ExitStack

import numpy as np

import concourse.bass as bass
import concourse.tile as tile
from concourse import bass_isa
from concourse import bacc, mybir
from concourse import bass_utils

B, S, D = 2, 2048, 2048
H, KVH = 16, 4
HD = 128               # head dim
GQ = 4                 # q heads per core
SL = GQ * HD           # 512: q-head slice width per core
NCORES = 8
SC = S // 512          # 4 s-chunks of 512
KC = D // 128          # 16 d-chunks of 128
ROPE_BASE = 10000.0
EPS = 1.1920929e-07
F32 = mybir.dt.float32
F32R = mybir.dt.float32r
AF = mybir.ActivationFunctionType

_COMPILED_NC = None
_LAST_IN_MAPS = None


def _build_body(tc):
    nc = tc.nc
    ctx = ExitStack()
    ctx.enter_context(nc.allow_low_precision(reason="fp32r matmul operand tiles"))

    xT = nc.dram_tensor("xT", [D, S], F32, kind="ExternalInput").ap()
    wqT = nc.dram_tensor("wqT", [D, SL], F32, kind="ExternalInput").ap()
    wkT = nc.dram_tensor("wkT", [D, HD], F32, kind="ExternalInput").ap()
    wvT = nc.dram_tensor("wvT", [D, HD], F32, kind="ExternalInput").ap()
    woA = nc.dram_tensor("woA", [SL, D], F32, kind="ExternalInput").ap()
    csd = nc.dram_tensor("csd", [128, S], F32, kind="ExternalInput").ap()
    snd = nc.dram_tensor("snd", [128, S], F32, kind="ExternalInput").ap()
    bqkd = nc.dram_tensor("bqkd", [128, GQ + 1], F32, kind="ExternalInput").ap()
    constd = nc.dram_tensor("constd", [257], F32, kind="ExternalInput").ap()
    identd = nc.dram_tensor("identd", [128, 128], F32, kind="ExternalInput").ap()
    sced = nc.dram_tensor("sced", [128, 2 * (GQ + 1)], F32, kind="ExternalInput").ap()
    resT = nc.dram_tensor("resT", [D, S], F32, kind="ExternalOutput").ap()

    persist = ctx.enter_context(tc.tile_pool(name="persist", bufs=1))
    xpool = ctx.enter_context(tc.tile_pool(name="xpool", bufs=9))
    wqpool = ctx.enter_context(tc.tile_pool(name="wqpool", bufs=4))
    wopool = ctx.enter_context(tc.tile_pool(name="wopool", bufs=3))
    bpool = ctx.enter_context(tc.tile_pool(name="bpool", bufs=2))
    rowp = ctx.enter_context(tc.tile_pool(name="rowp", bufs=2))
    expp = ctx.enter_context(tc.tile_pool(name="expp", bufs=3))
    otp = ctx.enter_context(tc.tile_pool(name="otp", bufs=2))
    resp = ctx.enter_context(tc.tile_pool(name="resp", bufs=2))
    vtp = ctx.enter_context(tc.tile_pool(name="vtp", bufs=2))
    psA = ctx.enter_context(tc.tile_pool(name="psA", bufs=3, space="PSUM"))
    psS = ctx.enter_context(tc.tile_pool(name="psS", bufs=2, space="PSUM"))
    psO = ctx.enter_context(tc.tile_pool(name="psO", bufs=2, space="PSUM"))
    psD = ctx.enter_context(tc.tile_pool(name="psD", bufs=1, space="PSUM"))

    # ---- tiny constants (DMA'd: memset cannot write fp32r); emission is
    # deferred until after the first projection pass so the critical-path
    # x/wq DMAs win queue priority at startup ----
    ones_col = persist.tile([128, 1], F32R, name="ones_col")
    bqcols = persist.tile([128, GQ + 1], F32, name="bqcols")
    sce = persist.tile([128, 2 * (GQ + 1)], F32, name="sce")

    def emit_const_dmas():
        nc.scalar.dma_start(ones_col, bass.AP(tensor=constd.tensor, offset=0,
                                              ap=[[1, 128], [1, 1]]).bitcast(F32R))
        nc.scalar.dma_start(bqcols, bqkd)
        nc.scalar.dma_start(sce, sced)
        nc.scalar.dma_start(
            wk_sb, wkT.rearrange("(kc p) h -> p kc h", p=128).bitcast(F32R))
        nc.scalar.dma_start(
            wv_sb, wvT.rearrange("(kc p) h -> p kc h", p=128).bitcast(F32R))
        nc.scalar.dma_start(cs_sb[:, 0:1024], csd[:, 0:1024])
        nc.scalar.dma_start(cs_sb[:, 1024:2048], csd[:, 1024:2048])
        nc.scalar.dma_start(sn_sb[:, 0:1024], snd[:, 0:1024])
        nc.scalar.dma_start(sn_sb[:, 1024:2048], snd[:, 1024:2048])
        nc.scalar.dma_start(ident, identd.bitcast(F32R))

    # persistent big tiles (DMAs emitted inside the sc loop so the first
    # projection chunk's x/w DMAs win queue priority at startup)
    cs_sb = persist.tile([128, S], F32, name="cs_sb")
    sn_sb = persist.tile([128, S], F32, name="sn_sb")
    wk_sb = persist.tile([128, KC, HD], F32R, name="wk_sb")
    wv_sb = persist.tile([128, KC, HD], F32R, name="wv_sb")
    ident = persist.tile([128, 128], F32R, name="ident")

    qfin = [
        persist.tile([128, S], F32R, name=f"qfin{h}", tag=f"qfin{h}") for h in range(GQ)
    ]
    kfin = persist.tile([128, S], F32R, name="kfin")
    v_sb = [
        persist.tile([128, HD], F32R, name=f"vsb{i}", tag=f"vsb{i}") for i in range(KC)
    ]

    # ================= Stage B: bias, rms-norm, rope (per [128,512] slice) ====
    def stage_b(et, sc, psum_p):
        """et in 0..3 -> q head et;  et == 4 -> k."""
        is_q = et < GQ
        bias_col = bqcols[:, et : et + 1] if is_q else bqcols[:, GQ : GQ + 1]
        q_raw = bpool.tile([128, 512], F32, tag="qraw", bufs=2)
        nc.vector.tensor_scalar_add(q_raw, psum_p, bias_col)
        # sum of squares along head dim (partitions) on gpsimd; gain and eps
        # are folded into the rsqrt via host-precomputed per-partition
        # scale/bias: g*rsqrt(ss/HD + eps) == rsqrt(ss*A + B)
        # sq = (psum+bias) * q_raw == q_raw^2 without a same-tensor dual read
        # (tensor_mul(x, x) reads one address on both ports: ~4x slower)
        sq = bpool.tile([128, 512], F32, tag="sq", bufs=2)
        nc.vector.scalar_tensor_tensor(sq, psum_p, bias_col, q_raw,
                                       op0=mybir.AluOpType.add,
                                       op1=mybir.AluOpType.mult)
        ssr = bpool.tile([128, 512], F32, tag="ssr", bufs=1)
        nc.gpsimd.partition_all_reduce(ssr, sq, 128, bass_isa.ReduceOp.add)
        scale_sb = bpool.tile([128, 512], F32, tag="rb", bufs=2,
                              name=f"scl{et}_{sc}")
        nc.scalar.activation(scale_sb, ssr, AF.Abs_reciprocal_sqrt,
                             bias=sce[:, GQ + 1 + et : GQ + 2 + et],
                             scale=sce[:, et : et + 1])
        # rope: swap halves via sbuf->sbuf DMA (sn rows 64..127 hold -sin)
        sw = bpool.tile([128, 512], F32, tag="sw", bufs=2)
        nc.sync.dma_start(sw[0:64, :], q_raw[64:128, :])
        nc.sync.dma_start(sw[64:128, :], q_raw[0:64, :])
        t1 = bpool.tile([128, 512], F32, tag="t1", bufs=2)
        nc.vector.tensor_mul(t1, q_raw, cs_sb[:, sc * 512 : (sc + 1) * 512])
        t2 = bpool.tile([128, 512], F32, tag="t2", bufs=1)
        nc.vector.tensor_mul(t2, sw, sn_sb[:, sc * 512 : (sc + 1) * 512])
        nc.vector.tensor_add(t1, t1, t2)
        dst = qfin[et] if is_q else kfin
        nc.vector.tensor_mul(dst[:, sc * 512 : (sc + 1) * 512], t1, scale_sb)

    # ================= Stage C: attention | Stage D: output proj =============
    def stage_cd(qc):
        otp_tiles = {}
        for h in range(GQ):
            nblk = 4 * (qc + 1)
            psum_o = psO.tile([128, 512], F32, tag="pO", name=f"pso{qc}_{h}")
            psum_d = psD.tile([1, 512], F32, tag="pD", name=f"psd{qc}_{h}")
            pend = None  # software-pipeline PV/denom one block behind
            for kt in range(nblk):
                ps_s = psS.tile([128, 512], F32, tag="pS", name=f"pss{qc}_{h}_{kt}")
                nc.tensor.matmul(
                    ps_s,
                    kfin[:, kt * 128 : (kt + 1) * 128],
                    qfin[h][:, qc * 512 : (qc + 1) * 512],
                    start=True, stop=True,
                )
                exp_s = expp.tile([128, 512], F32R, tag="exp")
                nc.scalar.activation(exp_s, ps_s, AF.Exp)
                if kt >= qc * 4:  # diagonal block: causal mask (keep q >= k)
                    nc.gpsimd.affine_select(
                        out=exp_s, in_=exp_s,
                        pattern=[[1, 512]],
                        compare_op=mybir.AluOpType.is_ge,
                        fill=0.0,
                        base=qc * 512 - kt * 128,
                        channel_multiplier=-1,
                    )
                if pend is not None:
                    pkt, pexp = pend
                    nc.tensor.matmul(psum_o, v_sb[pkt], pexp,
                                     start=(pkt == 0), stop=False)
                    nc.tensor.matmul(psum_d, ones_col, pexp,
                                     start=(pkt == 0), stop=False)
                pend = (kt, exp_s)
            pkt, pexp = pend
            nc.tensor.matmul(psum_o, v_sb[pkt], pexp, start=(pkt == 0), stop=True)
            nc.tensor.matmul(psum_d, ones_col, pexp, start=(pkt == 0), stop=True)
            # normalize: O.T = O'.T * (1/denom) broadcast across partitions
            rf_row = rowp.tile([1, 512], F32, tag="rfr", bufs=2)
            nc.vector.reciprocal_approx_fast(rf_row, psum_d)
            rb = bpool.tile([128, 512], F32, tag="rb", bufs=2)
            nc.gpsimd.partition_broadcast(rb, rf_row)
            ot = otp.tile([128, 512], F32R, tag=f"ot{h}", bufs=2)
            nc.vector.tensor_mul(ot, psum_o, rb)
            otp_tiles[h] = ot

        # Stage D for this qc: res.T[e, qc] = sum_h woA_h.T @ O_h.T
        for etg in range(8):
            wo_t = wopool.tile([128, GQ, 256], F32R, name=f"wo{qc}_{etg}", tag="wo")
            nc.sync.dma_start(
                wo_t,
                bass.AP(tensor=woA.tensor, offset=etg * 256,
                        ap=[[D, 128], [128 * D, GQ], [1, 256]]).bitcast(F32R),
            )
            r = resp.tile([128, 2, 512], F32, tag="res")
            for e2 in range(2):
                ps_res = psA.tile([128, 512], F32, tag="pA",
                                  name=f"psres{qc}_{etg}_{e2}")
                for h in range(GQ):
                    nc.tensor.matmul(
                        ps_res, wo_t[:, h, e2 * 128 : (e2 + 1) * 128],
                        otp_tiles[h],
                        start=(h == 0), stop=(h == GQ - 1),
                    )
                nc.vector.tensor_copy(r[:, e2, :], ps_res)
            nc.sync.dma_start(
                bass.AP(tensor=resT.tensor,
                        offset=etg * 2 * 128 * S + qc * 512,
                        ap=[[S, 128], [128 * S, 2], [1, 512]]),
                r,
            )


    # ================= Stage A: projections ===================================
    def emit_xq_dma(xt, sc, kp, eng):
        eng.dma_start(
            xt,
            bass.AP(
                tensor=xT.tensor,
                offset=kp * 256 * S + sc * 512,
                ap=[[S, 128], [128 * S, 2], [1, 512]],
            ).bitcast(F32R),
        )

    def make_wq(sc, pi, kq, eng):
        wt = wqpool.tile([128, 4, 256], F32R,
                         name=f"wq{sc}_{pi}_{kq}", tag="wq", bufs=4)
        eng.dma_start(
            wt,
            bass.AP(tensor=wqT.tensor,
                    offset=kq * 512 * SL + pi * 256,
                    ap=[[SL, 128], [128 * SL, 4], [1, 256]],
                    ).bitcast(F32R),
        )
        return wt

    rings = (nc.sync, nc.scalar)
    pending_vts = []

    def flush_vts():
        while pending_vts:
            vt, vsc = pending_vts.pop(0)
            for j in range(4):
                stile = vsc * 4 + j
                pst = psS.tile([128, 128], F32R, tag="pS", name=f"pst{vsc}_{j}")
                nc.tensor.transpose(pst, vt[:, j * 128 : (j + 1) * 128], ident)
                nc.vector.tensor_copy(v_sb[stile], pst)

    for sc in range(SC):
        flush_vts()
        xq = []
        wq0 = {}
        if sc == 0:
            # startup: finest-priority interleave so the first matmuls can
            # begin after ~0.75 MB of traffic, split across both rings
            xt = xpool.tile([128, 2, 512], F32R, name="xq0_0", tag="xq", bufs=9)
            for c2 in range(2):
                nc.sync.dma_start(
                    xt[:, c2, :],
                    bass.AP(tensor=xT.tensor, offset=c2 * 128 * S,
                            ap=[[S, 128], [1, 512]]).bitcast(F32R),
                )
            xq.append(xt)
            for kq in range(4):
                wq0[kq] = make_wq(0, 0, kq, nc.scalar)
            for kp in range(1, 8):
                xt = xpool.tile([128, 2, 512], F32R, name=f"xq0_{kp}", tag="xq",
                                bufs=9)
                emit_xq_dma(xt, 0, kp, nc.sync)
                xq.append(xt)
        else:
            for kp in range(8):
                xt = xpool.tile([128, 2, 512], F32R, name=f"xq{sc}_{kp}", tag="xq",
                                bufs=9)
                emit_xq_dma(xt, sc, kp, nc.sync)
                xq.append(xt)

        # passes: q-head pairs (0,1), (2,3), then (k, vT)
        for pi, grp in enumerate(((0, 1), (2, 3), (GQ, GQ + 1))):
            psums = {}
            for et in grp:
                psums[et] = psA.tile([128, 512], F32, tag="pA",
                                     name=f"psA{sc}_{pi}_{et}")
            wt = None
            for kq in range(4):
                if pi < 2:
                    if sc == 0 and pi == 0:
                        wt = wq0[kq]
                    else:
                        wt = make_wq(sc, pi, kq, nc.sync)
                for kc4 in range(4):
                    kc = kq * 4 + kc4
                    start = kc == 0
                    stop = kc == KC - 1
                    xsl = xq[kc // 2][:, kc % 2, :]
                    for ei, et in enumerate(grp):
                        if et < GQ:
                            lhsT = wt[:, kc4, ei * 128 : (ei + 1) * 128]
                        elif et == GQ:
                            lhsT = wk_sb[:, kc, :]
                        else:
                            lhsT = wv_sb[:, kc, :]
                        nc.tensor.matmul(psums[et], lhsT, xsl,
                                         start=start, stop=stop)
            if sc == 0 and pi == 0:
                emit_const_dmas()
            for et in grp:
                if et <= GQ:
                    stage_b(et, sc, psums[et])
                else:
                    # vT chunk -> sbuf; PE-transposes deferred to the next sc
                    # so they never stall the pass-boundary PE stream
                    vt = vtp.tile([128, 512], F32R, tag="vt", bufs=2)
                    nc.vector.tensor_copy(vt, psums[et])
                    pending_vts.append((vt, sc))

    flush_vts()
    for qc in range(SC):
        stage_cd(qc)

    ctx.close()


def _build():
    global _COMPILED_NC
    if _COMPILED_NC is not None:
        return _COMPILED_NC
    nc = bacc.Bacc("TRN2", target_bir_lowering=False, debug=False,
                   num_devices=NCORES)
    with tile.TileContext(nc) as tc:
        _build_body(tc)
    nc.compile()
    _COMPILED_NC = nc
    return nc


def _rope_tables():
    inv_freq = 1.0 / (ROPE_BASE ** (np.arange(0, HD, 2, dtype=np.float64) / HD))
    t = np.arange(S, dtype=np.float64)
    freqs = np.outer(t, inv_freq)          # [S, 64]
    cos = np.cos(freqs).T.astype(np.float32)   # [64, S]
    sin = np.sin(freqs).T.astype(np.float32)
    cs = np.concatenate([cos, cos], axis=0)    # [128, S]
    sn = np.concatenate([sin, -sin], axis=0)   # [128, S] (bottom half negated)
    return np.ascontiguousarray(cs), np.ascontiguousarray(sn)


def kernel(x, Wq, bq, Wk, bk, Wv, bv, Wo, bo, q_gain):
    x = np.asarray(x, np.float32)
    Wq = np.asarray(Wq, np.float32)
    bq = np.asarray(bq, np.float32)
    Wk = np.asarray(Wk, np.float32)
    bk = np.asarray(bk, np.float32)
    Wv = np.asarray(Wv, np.float32)
    bv = np.asarray(bv, np.float32)
    Wo = np.asarray(Wo, np.float32)
    bo = np.asarray(bo, np.float32)
    q_gain = np.asarray(q_gain, np.float32)

    cs, sn = _rope_tables()
    const_arr = np.concatenate([
        np.ones(128, np.float32),
        np.full(128, float(HD) ** -0.5, np.float32),
        np.array([EPS], np.float32),
    ])
    ident_arr = np.eye(128, dtype=np.float32)

    def sced_arr(g):
        gg = np.concatenate([g.astype(np.float64), [HD ** -0.5]])
        a = 1.0 / (HD * gg * gg)
        b = EPS / (gg * gg)
        row = np.concatenate([a, b]).astype(np.float32)   # [2*(GQ+1)]
        return np.ascontiguousarray(np.broadcast_to(row, (128, row.size)))

    in_maps = []
    for c in range(NCORES):
        b, g = divmod(c, KVH)
        sl = slice(g * SL, (g + 1) * SL)
        hs = slice(g * HD, (g + 1) * HD)
        in_maps.append({
            "xT": np.ascontiguousarray(x[b].T),
            "wqT": np.ascontiguousarray(Wq[sl, :].T),
            "wkT": np.ascontiguousarray(Wk[hs, :].T),
            "wvT": np.ascontiguousarray(Wv[hs, :].T),
            "woA": np.ascontiguousarray(Wo[:, sl].T),
            "csd": cs, "snd": sn,
            "bqkd": np.ascontiguousarray(np.concatenate(
                [bq[sl].reshape(GQ, HD).T, bk[hs].reshape(1, HD).T], axis=1)),
            "sced": sced_arr(q_gain[g * GQ : (g + 1) * GQ]),
            "constd": const_arr, "identd": ident_arr,
        })

    global _LAST_IN_MAPS
    _LAST_IN_MAPS = in_maps
    nc = _build()
    res = bass_utils.run_bass_kernel_spmd(nc, in_maps, core_ids=list(range(NCORES)))

    # v-bias and o-bias folded on host: attention rows sum to 1, so +bv
    # passes through to O exactly; res += bv_rep @ Wo.T + bo.
    bv_rep = np.repeat(bv.reshape(KVH, HD), H // KVH, axis=0).reshape(-1)
    host_const = (Wo @ bv_rep + bo).astype(np.float32)

    out = np.zeros((B, S, D), np.float32)
    for c in range(NCORES):
        b = c // KVH
        out[b] += res.results[c]["resT"].T
    out += host_const[None, None, :]
    return out



# revision 6
# speedup vs baseline: 1.1073x; 1.1073x over previous
"""Trainium2 Bass kernel for GQA attention (B=2, S=2048, D=2048, H=16, KVH=4).

Sharding: 8 cores = (batch b in {0,1}) x (kv-group g in {0..3}).
Core c = b*4 + g computes q-heads 4g..4g+3 against kv-head g for batch b,
producing a partial output projection res_partial.T = [e=2048, s=2048] in
bf16; host sums the 4 partials per batch.

v2 design notes (vs the f32r baseline at ~410us):
  - TRN2's PE p-states (0.65 -> 1.2 -> 2.4 GHz, 3us continuous ramp) punish
    every stall; the whole structure exists to keep the PE stream gap-free.
  - All matmuls in bf16 (psums f32): full 1 cyc/row at any free size, lower
    power (less 50% utilization throttling), half the DMA/SBUF footprint.
    Error budget ~1% vs the 2e-2 gate.
  - Weights fully resident in SBUF (no per-chunk wq/wo reloads).
  - RMS-norm partition reduce + broadcasts on the PE (tiny ones-matmuls)
    instead of gpsimd (2.5us/op there).
  - Causal masks are precomputed bf16 0/1 tiles applied by DVE multiply
    (affine_select on gpsimd was 0.6us + heavy semaphore cost).
  - exp on ACT over [128,1024] PSUM groups (amortizes the 352-cycle fixed
    overhead); scores/PV/denominator pipelined one group behind so the exp
    latency never blocks the PE.
  - Softmax denominator stays on the PE (ones-column matmul): cheaper there
    than any partition-reduction alternative.
  - Stage D (o-proj) accumulates in PSUM, copied to bf16 SBUF by DVE (2x
    mode) and DMA'd out as bf16 partials.
"""

import sys

sys.path.insert(0, "/opt/trn_rl_repo")

from contextlib import ExitStack

import numpy as np
import ml_dtypes

import concourse.bass as bass
import concourse.tile as tile
from concourse import bacc, mybir
from concourse import bass_utils

B, S, D = 2, 2048, 2048
H, KVH = 16, 4
HD = 128               # head dim
GQ = 4                 # q heads per core
SL = GQ * HD           # 512: q-head slice width per core
NCORES = 8
SC = S // 512          # 4 s-chunks of 512
KC = D // 128          # 16 d-chunks of 128
ROPE_BASE = 10000.0
EPS = 1.1920929e-07
F32 = mybir.dt.float32
F32R = mybir.dt.float32r
BF16 = mybir.dt.bfloat16
AF = mybir.ActivationFunctionType
BF = ml_dtypes.bfloat16

_COMPILED_NC = None
_LAST_IN_MAPS = None


def _build_body(tc):
    nc = tc.nc
    ctx = ExitStack()
    ctx.enter_context(nc.allow_low_precision(reason="bf16 datapath"))

    xT = nc.dram_tensor("xT", [D, S], BF16, kind="ExternalInput").ap()
    wqT = nc.dram_tensor("wqT", [D, SL], BF16, kind="ExternalInput").ap()
    wkT = nc.dram_tensor("wkT", [D, HD], BF16, kind="ExternalInput").ap()
    wvT = nc.dram_tensor("wvT", [D, HD], BF16, kind="ExternalInput").ap()
    woB = nc.dram_tensor("woB", [SL, D], BF16, kind="ExternalInput").ap()
    csd = nc.dram_tensor("csd", [128, S], BF16, kind="ExternalInput").ap()
    snd = nc.dram_tensor("snd", [128, S], BF16, kind="ExternalInput").ap()
    bqkd = nc.dram_tensor("bqkd", [128, GQ + 1], F32, kind="ExternalInput").ap()
    constd = nc.dram_tensor("constd", [257], F32, kind="ExternalInput").ap()
    sced = nc.dram_tensor("sced", [128, 2 * (GQ + 1)], F32, kind="ExternalInput").ap()
    # bf16 consts: [ident(128) | ones_col(1) | masks(4*512)]
    cbd = nc.dram_tensor("cbd", [128, 128 + 1 + 4 * 512], BF16,
                         kind="ExternalInput").ap()
    resT = nc.dram_tensor("resT", [D, S], BF16, kind="ExternalOutput").ap()

    persist = ctx.enter_context(tc.tile_pool(name="persist", bufs=1))
    xpool = ctx.enter_context(tc.tile_pool(name="xpool", bufs=10))
    bpool = ctx.enter_context(tc.tile_pool(name="bpool", bufs=2))
    rowp = ctx.enter_context(tc.tile_pool(name="rowp", bufs=2))
    expp = ctx.enter_context(tc.tile_pool(name="expp", bufs=3))
    otp = ctx.enter_context(tc.tile_pool(name="otp", bufs=2))
    resp = ctx.enter_context(tc.tile_pool(name="resp", bufs=4))
    vtp = ctx.enter_context(tc.tile_pool(name="vtp", bufs=2))
    # PSUM: exactly 8 banks. psS holds projection-pass pairs, score groups,
    # and stage-D accumulators; psO holds rms broadcasts, v-transposes and
    # attention accumulators; psD holds row vectors + recip broadcasts.
    psS = ctx.enter_context(tc.tile_pool(name="psS", bufs=2, space="PSUM"))
    psO = ctx.enter_context(tc.tile_pool(name="psO", bufs=2, space="PSUM"))
    psD = ctx.enter_context(tc.tile_pool(name="psD", bufs=2, space="PSUM"))

    # ---- persistent tiles ----
    cs_sb = persist.tile([128, S], BF16, name="cs_sb")
    sn_sb = persist.tile([128, S], BF16, name="sn_sb")
    wq_sb = {}
    for kq in range(4):
        for pi in range(2):
            wq_sb[(kq, pi)] = persist.tile([128, 4, 256], BF16,
                                           name=f"wq{kq}_{pi}")
    wk_sb = persist.tile([128, KC, HD], BF16, name="wk_sb")
    wv_sb = persist.tile([128, KC, HD], BF16, name="wv_sb")
    wo_sb = [persist.tile([128, GQ, 256], BF16, name=f"wo{e}") for e in range(8)]
    ident = persist.tile([128, 128], BF16, name="ident")
    ones_col = persist.tile([128, 1], BF16, name="ones_col")
    maskt = persist.tile([128, 4 * 512], BF16, name="maskt")
    onesr = persist.tile([1, 128], F32R, name="onesr")
    bqcols = persist.tile([128, GQ + 1], F32, name="bqcols")
    sce = persist.tile([128, 2 * (GQ + 1)], F32, name="sce")

    qfin = [persist.tile([128, S], BF16, name=f"qfin{h}") for h in range(GQ)]
    kfin = persist.tile([128, S], BF16, name="kfin")
    v_sb = [persist.tile([128, HD], BF16, name=f"vsb{i}") for i in range(KC)]

    def emit_const_dmas():
        # small/late-needed constants; emitted after the first projection
        # pass so the critical x/wq DMAs win queue priority at startup
        nc.scalar.dma_start(bqcols, bqkd)
        nc.scalar.dma_start(sce, sced)
        nc.scalar.dma_start(
            onesr, bass.AP(tensor=constd.tensor, offset=0,
                           ap=[[257, 1], [1, 128]]).bitcast(F32R))
        nc.scalar.dma_start(ident, cbd[:, 0:128])
        nc.scalar.dma_start(ones_col, cbd[:, 128:129])
        nc.scalar.dma_start(cs_sb, csd)
        nc.sync.dma_start(sn_sb, snd)
        nc.scalar.dma_start(maskt, cbd[:, 129:129 + 4 * 512])
        nc.scalar.dma_start(
            wk_sb, wkT.rearrange("(kc p) h -> p kc h", p=128))
        nc.scalar.dma_start(
            wv_sb, wvT.rearrange("(kc p) h -> p kc h", p=128))
        for e in range(8):
            nc.scalar.dma_start(
                wo_sb[e],
                bass.AP(tensor=woB.tensor, offset=e * 256,
                        ap=[[D, 128], [128 * D, GQ], [1, 256]]))

    # ================= Stage B: bias, rms-norm, rope (per [128,512] slice) ===
    pending_b = []

    def stage_b(et, sc, psum_half):
        """et in 0..3 -> q head et;  et == 4 -> k. Returns a closure that
        emits the stage-B chain (deferred one pass so its PE ops never stall
        the projection stream)."""
        is_q = et < GQ
        bias_col = bqcols[:, et: et + 1] if is_q else bqcols[:, GQ: GQ + 1]

        def emit():
            q_raw = bpool.tile([128, 512], BF16, tag="qraw", bufs=2,
                               name=f"qraw{et}_{sc}")
            nc.scalar.add(q_raw, psum_half, bias_col)
            sq = bpool.tile([128, 512], BF16, tag="sq", bufs=2,
                            name=f"sq{et}_{sc}")
            nc.scalar.activation(sq, psum_half, AF.Square, bias=bias_col)
            ssq = psD.tile([1, 512], F32, tag="pD", name=f"ssq{et}_{sc}")
            nc.tensor.matmul(ssq, ones_col, sq, start=True, stop=True)
            # gain & eps folded: g*rsqrt(ss/HD + eps) == rsqrt(ss*A + B)
            scale_row = bpool.tile([1, 512], F32R, tag="srow", bufs=2,
                                   name=f"srow{et}_{sc}")
            nc.scalar.activation(scale_row, ssq, AF.Abs_reciprocal_sqrt,
                                 bias=sce[0:1, GQ + 1 + et: GQ + 2 + et],
                                 scale=sce[0:1, et: et + 1])
            bc = psO.tile([128, 512], F32, tag="pO", name=f"bc{et}_{sc}")
            nc.tensor.matmul(bc, onesr, scale_row, start=True, stop=True)
            # rope: swap halves via sbuf->sbuf DMA (sn rows 64..127 hold -sin)
            sw = bpool.tile([128, 512], BF16, tag="sw", bufs=2,
                            name=f"sw{et}_{sc}")
            nc.sync.dma_start(sw[0:64, :], q_raw[64:128, :])
            nc.sync.dma_start(sw[64:128, :], q_raw[0:64, :])
            t1 = bpool.tile([128, 512], BF16, tag="t1", bufs=2,
                            name=f"t1_{et}_{sc}")
            nc.vector.tensor_mul(t1, q_raw, cs_sb[:, sc * 512: (sc + 1) * 512])
            t2 = bpool.tile([128, 512], BF16, tag="t2", bufs=2,
                            name=f"t2_{et}_{sc}")
            nc.vector.tensor_mul(t2, sw, sn_sb[:, sc * 512: (sc + 1) * 512])
            nc.vector.tensor_add(t1, t1, t2)
            dst = qfin[et] if is_q else kfin
            nc.vector.tensor_mul(dst[:, sc * 512: (sc + 1) * 512], t1, bc)

        return emit

    def flush_b():
        while pending_b:
            pending_b.pop(0)()

    # ================= Stage A: projections ==================================
    pending_vts = []

    def flush_vts():
        while pending_vts:
            vt, vsc = pending_vts.pop(0)
            for j in range(4):
                stile = vsc * 4 + j
                pst = psO.tile([128, 128], BF16, tag="pO", name=f"pst{stile}")
                nc.tensor.transpose(pst, vt[:, j * 128: (j + 1) * 128], ident)
                nc.vector.tensor_copy(v_sb[stile], pst)

    def emit_xq_dma(xt, sc, kp, eng):
        eng.dma_start(
            xt,
            bass.AP(
                tensor=xT.tensor,
                offset=kp * 256 * S + sc * 512,
                ap=[[S, 128], [128 * S, 2], [1, 512]],
            ),
        )

    for sc in range(SC):
        flush_vts()
        xq = []
        if sc == 0:
            # startup: finest-priority interleave; x on sync ring, wq on
            # scalar ring so the first matmuls can begin ASAP
            xt = xpool.tile([128, 2, 512], BF16, name="xq0_0", tag="xq")
            for c2 in range(2):
                nc.sync.dma_start(
                    xt[:, c2, :],
                    bass.AP(tensor=xT.tensor, offset=c2 * 128 * S,
                            ap=[[S, 128], [1, 512]]),
                )
            xq.append(xt)
            for kq in range(4):
                for pi in range(2):
                    nc.scalar.dma_start(
                        wq_sb[(kq, pi)],
                        bass.AP(tensor=wqT.tensor,
                                offset=kq * 512 * SL + pi * 256,
                                ap=[[SL, 128], [128 * SL, 4], [1, 256]]),
                    )
            for kp in range(1, 8):
                xt = xpool.tile([128, 2, 512], BF16, name=f"xq0_{kp}", tag="xq")
                emit_xq_dma(xt, 0, kp, nc.sync)
                xq.append(xt)
        else:
            for kp in range(8):
                xt = xpool.tile([128, 2, 512], BF16, name=f"xq{sc}_{kp}",
                                tag="xq")
                emit_xq_dma(xt, sc, kp, nc.sync)
                xq.append(xt)

        # passes: q-head pairs (0,1), (2,3), then (k, vT)
        for pi, grp in enumerate(((0, 1), (2, 3), (GQ, GQ + 1))):
            ps = psS.tile([128, 1024], F32, tag="pS", name=f"pa{sc}_{pi}")
            for kq in range(4):
                for kc4 in range(4):
                    kc = kq * 4 + kc4
                    start = kc == 0
                    stop = kc == KC - 1
                    xsl = xq[kc // 2][:, kc % 2, :]
                    for ei, et in enumerate(grp):
                        if et < GQ:
                            lhsT = wq_sb[(kq, pi)][:, kc4,
                                                   ei * 128: (ei + 1) * 128]
                        elif et == GQ:
                            lhsT = wk_sb[:, kc, :]
                        else:
                            lhsT = wv_sb[:, kc, :]
                        nc.tensor.matmul(ps[:, ei * 512: (ei + 1) * 512],
                                         lhsT, xsl, start=start, stop=stop)
            if sc == 0 and pi == 0:
                emit_const_dmas()
            flush_b()
            for ei, et in enumerate(grp):
                half = ps[:, ei * 512: (ei + 1) * 512]
                if et <= GQ:
                    pending_b.append(stage_b(et, sc, half))
                else:
                    # vT chunk -> bf16 sbuf; PE-transposes deferred to the
                    # next sc so they never stall the pass-boundary stream
                    vt = vtp.tile([128, 512], BF16, tag="vt", bufs=2,
                                  name=f"vt{sc}")
                    nc.scalar.copy(vt, half)
                    pending_vts.append((vt, sc))

    flush_b()
    flush_vts()

    # ================= Stage C: attention | Stage D: output proj =============
    # Flat software pipeline over (qc, head, kt-pair-group) work items.
    # PV/denominator run one group behind their exp; the normalization chain
    # runs three behind; stage D for qc is emitted three groups into qc+1.

    def groups_for(qc):
        nblk = 4 * (qc + 1)
        diag = [(4 * qc, 4 * qc + 1), (4 * qc + 2, 4 * qc + 3)]
        off = [(k, k + 1) for k in range(0, 4 * qc, 2)]
        return diag + off

    seq = []
    pending_D = None
    for qc in range(SC):
        gitems = []
        for h in range(GQ):
            gl = groups_for(qc)
            for gi, pair in enumerate(gl):
                gitems.append((qc, h, pair, gi == 0, gi == len(gl) - 1))
        for idx, it in enumerate(gitems):
            seq.append(("g",) + it)
            if idx == 2 and pending_D is not None:
                seq.append(("D", pending_D))
                pending_D = None
        pending_D = qc
    seq.append(("D", pending_D))

    psum_o = {}
    psum_d = {}
    ot_tiles = {}
    deferred = []  # (due_step, fn), kept in schedule order

    def flush_due(s):
        while deferred and deferred[0][0] <= s:
            deferred.pop(0)[1]()

    def flush_all():
        while deferred:
            deferred.pop(0)[1]()

    def make_pv(qc, h, pair, ex, first, last):
        def emit():
            po = psum_o[(qc, h)]
            pd = psum_d[(qc, h)]
            for j, kt in enumerate(pair):
                st = first and j == 0
                sp = last and j == 1
                exsl = ex[:, j * 512: (j + 1) * 512]
                nc.tensor.matmul(po, v_sb[kt], exsl, start=st, stop=sp)
                nc.tensor.matmul(pd, ones_col, exsl, start=st, stop=sp)
        return emit

    def make_norm(qc, h):
        def emit():
            rf = rowp.tile([1, 512], F32, tag="rf", bufs=2,
                           name=f"rf{qc}_{h}")
            nc.vector.reciprocal_approx_fast(rf, psum_d[(qc, h)])
            rfb = rowp.tile([1, 512], BF16, tag="rfb", bufs=2,
                            name=f"rfb{qc}_{h}")
            nc.vector.tensor_copy(rfb, rf)
            rb_ps = psD.tile([128, 512], F32, tag="pD", name=f"rbp{qc}_{h}")
            # maskt row 0 of the t=0 tile is all-ones bf16: reuse as [1,128]
            nc.tensor.matmul(rb_ps, maskt[0:1, 0:128], rfb,
                             start=True, stop=True)
            rb = bpool.tile([128, 512], BF16, tag="rb", bufs=2,
                            name=f"rb{qc}_{h}")
            nc.scalar.copy(rb, rb_ps)
            ot = otp.tile([128, 512], BF16, tag=f"ot{h}", name=f"ot{qc}_{h}")
            nc.vector.tensor_mul(ot, psum_o[(qc, h)], rb)
            ot_tiles[(qc, h)] = ot
        return emit

    def emit_stage_d(qc):
        flush_all()
        for etg in range(8):
            ps = psS.tile([128, 1024], F32, tag="pS", name=f"psd{qc}_{etg}")
            for e2 in range(2):
                for h in range(GQ):
                    nc.tensor.matmul(
                        ps[:, e2 * 512: (e2 + 1) * 512],
                        wo_sb[etg][:, h, e2 * 128: (e2 + 1) * 128],
                        ot_tiles[(qc, h)],
                        start=(h == 0), stop=(h == GQ - 1),
                    )
            r = resp.tile([128, 2, 512], BF16, tag="res", name=f"r{qc}_{etg}")
            for e2 in range(2):
                nc.vector.tensor_copy(r[:, e2, :],
                                      ps[:, e2 * 512: (e2 + 1) * 512])
            eng = nc.sync if etg % 2 == 0 else nc.scalar
            eng.dma_start(
                bass.AP(tensor=resT.tensor,
                        offset=etg * 2 * 128 * S + qc * 512,
                        ap=[[S, 128], [128 * S, 2], [1, 512]]),
                r,
            )

    step = 0
    for it in seq:
        if it[0] == "g":
            _, qc, h, pair, first, last = it
            flush_due(step)
            if first:
                psum_o[(qc, h)] = psO.tile([128, 512], F32, tag="pO",
                                           name=f"pso{qc}_{h}")
                psum_d[(qc, h)] = psD.tile([1, 512], F32, tag="pD",
                                           name=f"psd_{qc}_{h}")
            ps = psS.tile([128, 1024], F32, tag="pS",
                          name=f"pss{qc}_{h}_{pair[0]}")
            qsl = qfin[h][:, qc * 512: (qc + 1) * 512]
            for j, kt in enumerate(pair):
                nc.tensor.matmul(ps[:, j * 512: (j + 1) * 512],
                                 kfin[:, kt * 128: (kt + 1) * 128], qsl,
                                 start=True, stop=True)
            ex = expp.tile([128, 1024], BF16, tag="exp",
                           name=f"ex{qc}_{h}_{pair[0]}")
            nc.scalar.activation(ex, ps, AF.Exp)
            t0 = pair[0] - 4 * qc
            if t0 >= 0:  # diagonal pair: causal mask (keep q >= k)
                nc.vector.tensor_mul(
                    ex, ex, maskt[:, t0 * 512: (t0 + 2) * 512])
            deferred.append((step + 1, make_pv(qc, h, pair, ex, first, last)))
            if last:
                deferred.append((step + 3, make_norm(qc, h)))
            step += 1
        else:
            emit_stage_d(it[1])

    ctx.close()


def _build():
    global _COMPILED_NC
    if _COMPILED_NC is not None:
        return _COMPILED_NC
    nc = bacc.Bacc("TRN2", target_bir_lowering=False, debug=False,
                   num_devices=NCORES)
    with tile.TileContext(nc) as tc:
        _build_body(tc)
    nc.compile()
    _COMPILED_NC = nc
    return nc


def _rope_tables():
    inv_freq = 1.0 / (ROPE_BASE ** (np.arange(0, HD, 2, dtype=np.float64) / HD))
    t = np.arange(S, dtype=np.float64)
    freqs = np.outer(t, inv_freq)          # [S, 64]
    cos = np.cos(freqs).T.astype(np.float32)   # [64, S]
    sin = np.sin(freqs).T.astype(np.float32)
    cs = np.concatenate([cos, cos], axis=0)    # [128, S]
    sn = np.concatenate([sin, -sin], axis=0)   # [128, S] (bottom half negated)
    return cs.astype(BF), sn.astype(BF)


def _mask_tiles():
    # maskt[p, t*512 + j] = 1 if causal keeps (k_local=p+128t, q_local=j)
    p = np.arange(128)[:, None]
    j = np.arange(512)[None, :]
    cols = [(j >= p + 128 * t).astype(np.float32) for t in range(4)]
    return np.concatenate(cols, axis=1).astype(BF)


def kernel(x, Wq, bq, Wk, bk, Wv, bv, Wo, bo, q_gain):
    x = np.asarray(x, np.float32)
    Wq = np.asarray(Wq, np.float32)
    bq = np.asarray(bq, np.float32)
    Wk = np.asarray(Wk, np.float32)
    bk = np.asarray(bk, np.float32)
    Wv = np.asarray(Wv, np.float32)
    bv = np.asarray(bv, np.float32)
    Wo = np.asarray(Wo, np.float32)
    bo = np.asarray(bo, np.float32)
    q_gain = np.asarray(q_gain, np.float32)

    cs, sn = _rope_tables()
    const_arr = np.concatenate([
        np.ones(128, np.float32),
        np.full(128, float(HD) ** -0.5, np.float32),
        np.array([EPS], np.float32),
    ])
    cb = np.concatenate([
        np.eye(128, dtype=np.float32).astype(BF),
        np.ones((128, 1), np.float32).astype(BF),
        _mask_tiles(),
    ], axis=1)

    def sced_arr(g):
        gg = np.concatenate([g.astype(np.float64), [HD ** -0.5]])
        a = 1.0 / (HD * gg * gg)
        b = EPS / (gg * gg)
        row = np.concatenate([a, b]).astype(np.float32)   # [2*(GQ+1)]
        return np.ascontiguousarray(np.broadcast_to(row, (128, row.size)))

    in_maps = []
    for c in range(NCORES):
        b, g = divmod(c, KVH)
        sl = slice(g * SL, (g + 1) * SL)
        hs = slice(g * HD, (g + 1) * HD)
        in_maps.append({
            "xT": np.ascontiguousarray(x[b].T.astype(BF)),
            "wqT": np.ascontiguousarray(Wq[sl, :].T.astype(BF)),
            "wkT": np.ascontiguousarray(Wk[hs, :].T.astype(BF)),
            "wvT": np.ascontiguousarray(Wv[hs, :].T.astype(BF)),
            "woB": np.ascontiguousarray(Wo[:, sl].T.astype(BF)),
            "csd": cs, "snd": sn,
            "bqkd": np.ascontiguousarray(np.concatenate(
                [bq[sl].reshape(GQ, HD).T, bk[hs].reshape(1, HD).T], axis=1)),
            "sced": sced_arr(q_gain[g * GQ: (g + 1) * GQ]),
            "constd": const_arr,
            "cbd": cb,
        })

    global _LAST_IN_MAPS
    _LAST_IN_MAPS = in_maps
    nc = _build()
    res = bass_utils.run_bass_kernel_spmd(nc, in_maps, core_ids=list(range(NCORES)))

    # v-bias and o-bias folded on host: attention rows sum to 1, so +bv
    # passes through to O exactly; res += bv_rep @ Wo.T + bo.
    bv_rep = np.repeat(bv.reshape(KVH, HD), H // KVH, axis=0).reshape(-1)
    host_const = (Wo @ bv_rep + bo).astype(np.float32)

    out = np.zeros((B, S, D), np.float32)
    for c in range(NCORES):
        b = c // KVH
        out[b] += res.results[c]["resT"].astype(np.float32).T
    out += host_const[None, None, :]
    return out
